# revision 1
# baseline (speedup 1.0000x reference)
"""Causal self-attention Trainium2 kernel (B=4, T=2048, D=1024, H=16).

Sharding: 8 cores = 4 batches x 2 head-groups (8 heads each). Each core
computes its batch's qkv projection restricted to its 8 heads, causal
attention for those heads, and a partial out-projection over its 512 ctx
channels. Host sums the two partials per batch and adds b_out.

Per-core layout choices (all matmuls bf16 with fp32 PSUM accumulation;
fp8 was tried for the q/k projection and rejected: ~2.6%/element e4m3
noise on q,k puts ~5% absolute noise on scores, and score noise passes
~1:1 into the output because the softmax numerator pairs it with random
v — measured 4.6% rel err):
  - The 1/sqrt(dk) scale is folded into Wq/bq.
  - qkT: per head-pair p, a q-tile [128, T] (head A rows 0:64, head B rows
    64:128) and a k-tile [128, T].
  - scoresT[s, t] blocks [128, 512]: lhsT=kT (K=64 rows), rhs=qT. Heads A/B
    run concurrently in disjoint PE row groups. Diagonal blocks only
    compute the causally needed t-range.
  - causal mask: after exp, the diagonal 128x128 square is multiplied by a
    {0,1} strict-triangle tile on the DVE (cheaper than the old identity
    matmul pair accumulating -30000 into PSUM).
  - softmax: no max-subtraction (scores are within +-10 by construction),
    exp on ScalarE PSUM->SBUF bf16.
  - ctx: v stored naturally [s, d] with a ones column appended per head
    (v_ext [128, 8*65]); lhsT=v_ext (M=65) so PSUM row 64 accumulates the
    softmax denominator. Normalize = reciprocal_approx_fast + gpsimd
    partition_broadcast + DVE mul into the bf16 ctxT copy.
  - out projection: ctxT pair-tiles [128, T] are the stationary operand
    against W_outT; output DMA'd bf16, partials summed on host in fp32
    (b_out added there).

Emission order: the prologue computes the i=0 projections ci-outer (first
matmul starts after ~0.8 MB of DMA) and all four i=0 attn pairs. Each
steady segment i emits the i+1 projections and ALL FOUR i+1 attn pairs
with the out-projection chunks of iteration i interleaved between pairs,
so the PE always has independent work while ScalarE exps and the
normalize chains drain, and the final segment is just out_proj(3).
"""

import math

import numpy as np
import ml_dtypes

B, T, C = 4, 2048, 1024
H, DK = 16, 64
NCORES = 8
TS = 128  # s-tile (partition granularity)
TSL = 512  # t free-dim tile (one PSUM bank of fp32)
BF16 = ml_dtypes.bfloat16


def build_program(C_sz=C, T_sz=T, n_pairs=4, num_devices=1):
    import concourse.mybir as mybir
    from concourse import bacc
    from concourse.tile import TileContext

    dt = mybir.dt
    f32 = dt.float32
    bf16 = dt.bfloat16
    AF = mybir.ActivationFunctionType

    n_ct = C_sz // 128  # contraction tiles for projections
    n_qk = 2 * n_pairs  # qk o-tiles (128 channels each)
    VW = n_pairs * 2 * DK  # v channels (natural order)
    n_tt = T_sz // TS
    n_it = T_sz // TSL
    JPI = TSL // TS  # s-tiles per i-tile (4)
    OW = min(TSL, C_sz)  # output column tile width
    n_oh = C_sz // OW  # output column halves
    # v_ext: per head [v (64 cols) | ones (64 cols)] so the ctx matmul
    # (M=128, same stream cost as M=65) replicates the softmax denominator
    # across 64 PSUM partitions — no gpsimd partition_broadcast needed
    VEW = n_pairs * 2 * 2 * DK

    nc = bacc.Bacc(
        "TRN2",
        target_bir_lowering=False,
        debug=False,
        num_devices=num_devices,
    )

    xT_d = nc.dram_tensor("xT", [C_sz, T_sz], bf16, kind="ExternalInput").ap()
    wqk_d = nc.dram_tensor("wqkT", [C_sz, n_qk * 128], bf16, kind="ExternalInput").ap()
    wv_d = nc.dram_tensor("wvT", [C_sz, VW], bf16, kind="ExternalInput").ap()
    bqk_d = nc.dram_tensor("bqk", [128, n_qk], f32, kind="ExternalInput").ap()
    bv_d = nc.dram_tensor("bv", [1, VW], bf16, kind="ExternalInput").ap()
    wo_d = nc.dram_tensor("woT", [n_pairs * 128, C_sz], bf16, kind="ExternalInput").ap()
    tri_d = nc.dram_tensor("trisq", [128, 2 * TS], bf16, kind="ExternalInput").ap()
    out_d = nc.dram_tensor("out", [T_sz, C_sz], bf16, kind="ExternalOutput").ap()

    with TileContext(nc) as tc:
        with (
            tc.tile_pool(name="const", bufs=1) as const_pool,
            tc.tile_pool(name="big", bufs=1) as big_pool,
            tc.tile_pool(name="attn", bufs=10) as attn_pool,
            tc.tile_pool(name="rinv", bufs=6) as rinv_pool,
            tc.tile_pool(name="outsb", bufs=6) as outsb_pool,
            tc.tile_pool(name="sc", bufs=2, space="PSUM") as sc_ps,
            tc.tile_pool(name="mm", bufs=4, space="PSUM") as mm_ps,
        ):
            # ---- weight/activation loads (first compute inputs first,
            # interleaved per contraction tile so the ci-outer prologue can
            # start after the first ~1 MB) ----
            xT_sb = []
            wqk_sb = []
            wv_sb = []
            for ci in range(n_ct):
                t = big_pool.tile([128, T_sz], bf16, tag=f"xT{ci}", name=f"xT{ci}")
                # the prologue only reads columns 0:TSL (i=0): load those
                # first so the ci-outer loop can start sooner; the rest of
                # xT streams in afterwards (first consumer is qk(.,1) in
                # segment 0, tens of us later)
                nc.sync.dma_start(t[:, 0:TSL], xT_d[ci * 128 : (ci + 1) * 128, 0:TSL])
                xT_sb.append(t)
                t = big_pool.tile(
                    [128, n_qk * 128], bf16, tag=f"wqk{ci}", name=f"wqk{ci}"
                )
                nc.sync.dma_start(t[:], wqk_d[ci * 128 : (ci + 1) * 128, :])
                wqk_sb.append(t)
                t = big_pool.tile([128, VW], bf16, tag=f"wv{ci}", name=f"wv{ci}")
                nc.sync.dma_start(t[:], wv_d[ci * 128 : (ci + 1) * 128, :])
                wv_sb.append(t)
            bqk_sb = const_pool.tile([128, n_qk], f32, tag="bqk", name="bqk")
            nc.sync.dma_start(bqk_sb[:], bqk_d)
            for ci in range(n_ct):
                nc.sync.dma_start(
                    xT_sb[ci][:, TSL:T_sz],
                    xT_d[ci * 128 : (ci + 1) * 128, TSL:T_sz],
                )
            bv_sb = const_pool.tile([1, VW], bf16, tag="bv", name="bv")
            nc.sync.dma_start(bv_sb[:], bv_d)
            bv_bc = const_pool.tile([128, VW], bf16, tag="bv_bc", name="bv_bc")
            nc.gpsimd.partition_broadcast(bv_bc[:], bv_sb[:])
            ones_bc = const_pool.tile([128, TSL], bf16, tag="ones_bc", name="ones_bc")
            nc.gpsimd.memset(ones_bc[:], 1.0)
            tri_sb = const_pool.tile([128, 2 * TS], bf16, tag="tri", name="tri")
            nc.sync.dma_start(tri_sb[:], tri_d)
            wo_sb = []
            for p in range(n_pairs):
                t = big_pool.tile([128, C_sz], bf16, tag=f"wo{p}", name=f"wo{p}")
                nc.sync.dma_start(t[:], wo_d[p * 128 : (p + 1) * 128, :])
                wo_sb.append(t)

            qkT_sb = [
                big_pool.tile([128, T_sz], bf16, tag=f"qkT{ot}", name=f"qkT{ot}")
                for ot in range(n_qk)
            ]
            vext_sb = [
                big_pool.tile([128, VEW], bf16, tag=f"vext{tt}", name=f"vext{tt}")
                for tt in range(n_tt)
            ]
            for tt in range(n_tt):
                # one-time: the ones half of every per-head v_ext block
                vx3 = vext_sb[tt][:].rearrange("p (h e) -> p h e", e=2 * DK)
                nc.gpsimd.memset(vx3[:, :, DK : 2 * DK], 1.0)
            ctxT_sb = [
                big_pool.tile([128, T_sz], bf16, tag=f"ctxT{p}", name=f"ctxT{p}")
                for p in range(n_pairs)
            ]

            def qk_copy(ot, i, ps):
                # (ps + bias) * 1.0 on the DVE: keeps the ScalarE free for
                # the softmax exps, which pace the attn inner loop
                nc.vector.scalar_tensor_tensor(
                    qkT_sb[ot][:, i * TSL : (i + 1) * TSL],
                    ps[:],
                    bqk_sb[:, ot : ot + 1],
                    ones_bc[:],
                    op0=mybir.AluOpType.add,
                    op1=mybir.AluOpType.mult,
                )

            def qk_mms(ot, i, ps, c0, c1):
                for ci in range(c0, c1):
                    nc.tensor.matmul(
                        ps[:],
                        lhsT=wqk_sb[ci][:, ot * 128 : (ot + 1) * 128],
                        rhs=xT_sb[ci][:, i * TSL : (i + 1) * TSL],
                        start=(ci == 0),
                        stop=(ci == n_ct - 1),
                    )

            def qk_units(ot, i):
                # a qk projection split into two ~1.7us filler units;
                # the PSUM tile is allocated lazily at emission time
                box = {}

                def head():
                    box["ps"] = mm_ps.tile([128, TSL], f32, tag="mm", name="mm")
                    qk_mms(ot, i, box["ps"], 0, n_ct // 2)

                def tail():
                    qk_mms(ot, i, box["ps"], n_ct // 2, n_ct)
                    qk_copy(ot, i, box["ps"])

                return [head, tail]

            def v_mms(tt, ps_ap, c0, c1):
                for ci in range(c0, c1):
                    nc.tensor.matmul(
                        ps_ap,
                        lhsT=xT_sb[ci][:, tt * TS : (tt + 1) * TS],
                        rhs=wv_sb[ci][:],
                        start=(ci == 0),
                        stop=(ci == n_ct - 1),
                        skip_group_check=True,
                    )

            def v_finish(tt, ps_ap):
                # ps_ap: [128, VW/2] fp32 PSUM access pattern (v channels
                # only; the ones half of each head block is memset once in
                # the prologue)
                vx3 = vext_sb[tt][:].rearrange("p (h e) -> p h e", e=2 * DK)
                nc.vector.scalar_tensor_tensor(
                    vx3[:, :, 0:DK],
                    ps_ap.rearrange("p (h e) -> p h e", e=DK),
                    1.0,
                    bv_bc[:].rearrange("p (h e) -> p h e", e=DK),
                    op0=mybir.AluOpType.mult,
                    op1=mybir.AluOpType.add,
                )

            def v_units(tt):
                box = {}

                def head():
                    box["ps"] = mm_ps.tile([128, VW], f32, tag="mm", name="mm")
                    v_mms(tt, box["ps"][:], 0, n_ct // 2)

                def tail():
                    v_mms(tt, box["ps"][:], n_ct // 2, n_ct)
                    v_finish(tt, box["ps"][:])

                return [head, tail]

            def out_chunk(i, c):
                tt, oh = JPI * i + c // n_oh, c % n_oh
                ps = mm_ps.tile([128, OW], f32, tag="mm", name="mm")
                for p in range(n_pairs):
                    nc.tensor.matmul(
                        ps[:],
                        lhsT=ctxT_sb[p][:, tt * TS : (tt + 1) * TS],
                        rhs=wo_sb[p][:, oh * OW : (oh + 1) * OW],
                        start=(p == 0),
                        stop=(p == n_pairs - 1),
                    )
                ob = outsb_pool.tile([128, OW], bf16, tag="outsb", name="outsb")
                nc.scalar.activation(ob[:], ps[:], AF.Identity)
                nc.sync.dma_start(
                    out_d[tt * TS : (tt + 1) * TS, oh * OW : (oh + 1) * OW],
                    ob[:],
                )

            def attn_pair(p, i, fillers=None):
                """Scores run two j-blocks ahead of the ctx consumers so the
                ctx->exp dependency never stalls an empty PE pipeline; one
                filler unit (projection / out-proj work) is emitted per ctx
                block to cover the ScalarE exp pacing."""
                qt, kt = qkT_sb[2 * p], qkT_sb[2 * p + 1]
                nj = JPI * (i + 1)
                ctxA = mm_ps.tile([128, TSL], f32, tag="mm", name="mm")
                ctxB = mm_ps.tile([128, TSL], f32, tag="mm", name="mm")
                tri3 = tri_sb[:].rearrange("p (c w) -> p c w", c=2)

                def scores_block(j):
                    diag = j >= JPI * i
                    pi = j - JPI * i if diag else 0
                    t0 = pi * TS  # first causally-live t column in this block
                    ps = sc_ps.tile([128, 2 * TSL], f32, tag="sc", name="sc")
                    nc.tensor.matmul(
                        ps[:, t0:TSL],
                        lhsT=kt[0:64, j * TS : (j + 1) * TS],
                        rhs=qt[0:64, i * TSL + t0 : (i + 1) * TSL],
                        start=True,
                        stop=True,
                        skip_group_check=True,
                    )
                    nc.tensor.matmul(
                        ps[:, TSL + t0 : 2 * TSL],
                        lhsT=kt[64:128, j * TS : (j + 1) * TS],
                        rhs=qt[64:128, i * TSL + t0 : (i + 1) * TSL],
                        start=True,
                        stop=True,
                        skip_group_check=True,
                    )
                    a = attn_pool.tile([128, 2 * TSL], bf16, tag="attn", name="attn")
                    a3 = a[:].rearrange("p (c w) -> p c w", c=2)
                    ps3 = ps[:].rearrange("p (c w) -> p c w", c=2)
                    nc.scalar.activation(a3[:, :, t0:TSL], ps3[:, :, t0:TSL], AF.Exp)
                    if diag:
                        # zero the below-diagonal triangle of the 128x128
                        # square (exp of unmasked scores is finite garbage)
                        nc.vector.tensor_mul(
                            a3[:, :, t0 : t0 + TS], a3[:, :, t0 : t0 + TS], tri3
                        )
                    return a, t0

                pend = [scores_block(j) for j in range(min(2, nj))]
                for j in range(nj):
                    a, t0 = pend.pop(0)
                    nc.tensor.matmul(
                        ctxA[:, t0:TSL],
                        lhsT=vext_sb[j][:, (2 * p) * 2 * DK : (2 * p + 1) * 2 * DK],
                        rhs=a[:, t0:TSL],
                        start=(j == 0),
                        stop=(j == nj - 1),
                    )
                    nc.tensor.matmul(
                        ctxB[:, t0:TSL],
                        lhsT=vext_sb[j][
                            :, (2 * p + 1) * 2 * DK : (2 * p + 2) * 2 * DK
                        ],
                        rhs=a[:, TSL + t0 : 2 * TSL],
                        start=(j == 0),
                        stop=(j == nj - 1),
                    )
                    if j + 2 < nj:
                        pend.append(scores_block(j + 2))
                    if fillers:
                        fillers.pop(0)()
                isl = slice(i * TSL, (i + 1) * TSL)
                for cps, rows in ((ctxA, slice(0, 64)), (ctxB, slice(64, 128))):
                    # PSUM rows 64:128 hold the denominator replicated 64x
                    # (ones half of v_ext). custom-DVE ops misread PSUM on
                    # hw: bounce via SBUF, then reciprocal + multiply.
                    rs = rinv_pool.tile([DK, TSL], f32, tag="rsum", name="rsum")
                    nc.vector.tensor_copy(rs[:], cps[DK : 2 * DK, :])
                    r = rinv_pool.tile([DK, TSL], f32, tag="rinv", name="rinv")
                    nc.vector.reciprocal_approx_fast(r[:], rs[:])
                    nc.vector.tensor_mul(ctxT_sb[p][rows, isl], cps[0:DK, :], r[:])

            # ---- prologue: i=0 projections, ci-outer so the first matmuls
            # only wait on the first DMA tiles. The first ci loop also does
            # the i=0 v projections (in halves of two score-pool PSUM tiles)
            # so the DMA-paced phase is dense enough to keep HAM warm ----
            pss = [mm_ps.tile([128, TSL], f32, tag="mm", name="mm") for _ in range(4)]
            vsc = [
                sc_ps.tile([128, 2 * TSL], f32, tag="sc", name="sc") for _ in range(2)
            ]
            vap = [vsc[tt // 2][:, (tt % 2) * TSL : (tt % 2) * TSL + VW]
                   for tt in range(JPI)]
            for ci in range(n_ct):
                for oi in range(4):
                    nc.tensor.matmul(
                        pss[oi][:],
                        lhsT=wqk_sb[ci][:, oi * 128 : (oi + 1) * 128],
                        rhs=xT_sb[ci][:, 0:TSL],
                        start=(ci == 0),
                        stop=(ci == n_ct - 1),
                    )
                for tt in range(JPI):
                    v_mms(tt, vap[tt], ci, ci + 1)
            for oi in range(4):
                qk_copy(oi, 0, pss[oi])
            for tt in range(JPI):
                v_finish(tt, vap[tt])
            pss = [mm_ps.tile([128, TSL], f32, tag="mm", name="mm") for _ in range(4)]
            for ci in range(n_ct):
                for oi in range(4):
                    ot = 4 + oi
                    nc.tensor.matmul(
                        pss[oi][:],
                        lhsT=wqk_sb[ci][:, ot * 128 : (ot + 1) * 128],
                        rhs=xT_sb[ci][:, 0:TSL],
                        start=(ci == 0),
                        stop=(ci == n_ct - 1),
                    )
            for oi in range(4):
                qk_copy(4 + oi, 0, pss[oi])

            def run_pairs(pairs, fillers):
                """Emit attn pairs with the filler units spread evenly over
                their ctx blocks (order-preserving; pads with no-ops)."""
                nblocks = sum(JPI * (ii + 1) for _, ii in pairs)
                k = len(fillers)
                spaced = []
                for bi in range(nblocks):
                    take = (bi * k) // nblocks != ((bi + 1) * k) // nblocks
                    spaced.append(fillers[(bi * k) // nblocks] if take else None)
                for p, ii in pairs:
                    nb = JPI * (ii + 1)
                    attn_pair(
                        p, ii, fillers=[(u or (lambda: None)) for u in spaced[:nb]]
                    )
                    spaced = spaced[nb:]

            # ---- main loop: segment i emits iteration i+1's projections and
            # attn pairs (plus, for i=0, iteration 0's pairs) with iteration
            # i's out-projection chunks interleaved as PE filler ----
            for i in range(n_it):
                if i + 1 < n_it:
                    for ot in (0, 1):
                        for u in qk_units(ot, i + 1):
                            u()
                    # new vext tiles must exist before any diag block of the
                    # new pairs consumes them: emit the v units directly
                    for tt in range(JPI * (i + 1), JPI * (i + 2)):
                        for u in v_units(tt):
                            u()
                    qk_fill = []
                    for ot in range(2, n_qk):
                        qk_fill.extend(qk_units(ot, i + 1))
                    out_fill = [
                        (lambda cc: lambda: out_chunk(i, cc))(c)
                        for c in range(2 * JPI)
                    ]
                    if i == 0:
                        # iteration 0's pairs first (out chunks of iteration 0
                        # must wait for all of them), then iteration 1's
                        run_pairs([(p, 0) for p in range(n_pairs)], qk_fill)
                        run_pairs([(p, 1) for p in range(n_pairs)], out_fill)
                    else:
                        run_pairs(
                            [(p, i + 1) for p in range(n_pairs)],
                            qk_fill + out_fill,
                        )
                else:
                    # final segment: run the pair-0..2 matmuls of all chunks
                    # first — they are independent of the last pair's
                    # normalize chain, so the PE stays busy (and HAM warm)
                    # while it drains; only the pair-3 matmuls wait on it.
                    fps = [
                        mm_ps.tile([128, OW], f32, tag="mm", name="mm")
                        for _ in range(4)
                    ]
                    fsc = [
                        sc_ps.tile([128, 2 * TSL], f32, tag="sc", name="sc")
                        for _ in range(2)
                    ]
                    faps = [t[:] for t in fps] + [
                        fsc[h][:, (c % 2) * TSL : (c % 2) * TSL + OW]
                        for h in range(2)
                        for c in range(2)
                    ]
                    for c in range(2 * JPI):
                        tt, oh = JPI * i + c // n_oh, c % n_oh
                        for p in range(n_pairs - 1):
                            nc.tensor.matmul(
                                faps[c],
                                lhsT=ctxT_sb[p][:, tt * TS : (tt + 1) * TS],
                                rhs=wo_sb[p][:, oh * OW : (oh + 1) * OW],
                                start=(p == 0),
                                stop=False,
                                skip_group_check=True,
                            )
                    for c in range(2 * JPI):
                        tt, oh = JPI * i + c // n_oh, c % n_oh
                        nc.tensor.matmul(
                            faps[c],
                            lhsT=ctxT_sb[n_pairs - 1][:, tt * TS : (tt + 1) * TS],
                            rhs=wo_sb[n_pairs - 1][:, oh * OW : (oh + 1) * OW],
                            start=False,
                            stop=True,
                            skip_group_check=True,
                        )
                        ob = outsb_pool.tile([128, OW], bf16, tag="outsb", name="outsb")
                        nc.scalar.activation(ob[:], faps[c], AF.Identity)
                        nc.sync.dma_start(
                            out_d[tt * TS : (tt + 1) * TS, oh * OW : (oh + 1) * OW],
                            ob[:],
                        )

    nc.compile()
    return nc


def make_tri_square(ts=TS):
    """[128, 2*ts] {0,1} keep-mask, duplicated per head: cell (s, t) = 0 iff
    s > t (strictly below the diagonal of the 128x128 square)."""
    s = np.arange(128)[:, None]
    t = np.arange(ts)[None, :]
    one = np.where(s > t, 0.0, 1.0).astype(np.float32)
    return np.concatenate([one, one], axis=1)


def make_core_inputs(x_b, W_qkv, b_qkv, W_out, heads, C_sz=C, T_sz=T):
    """Build the per-core input map (numpy, host-side)."""
    n_pairs = len(heads) // 2
    n_qk = 2 * n_pairs
    VW = len(heads) * DK
    xT = np.ascontiguousarray(x_b.T).astype(np.float32)
    wqk = np.empty((C_sz, n_qk * 128), np.float32)
    bqk = np.empty((128, n_qk), np.float32)
    wv = np.empty((C_sz, VW), np.float32)
    bv = np.empty((1, VW), np.float32)
    wo = np.empty((n_pairs * 128, C_sz), np.float32)
    for p in range(n_pairs):
        hA, hB = heads[2 * p], heads[2 * p + 1]
        # q tile (scaled by 1/sqrt(dk)=1/8), k tile
        for half, h in ((0, hA), (1, hB)):
            r0 = h * 3 * DK
            wqk[:, 2 * p * 128 + half * 64 : 2 * p * 128 + half * 64 + 64] = (
                W_qkv[r0 : r0 + DK].T / math.sqrt(DK)
            )
            bqk[half * 64 : half * 64 + 64, 2 * p] = b_qkv[r0 : r0 + DK] / math.sqrt(DK)
            wqk[:, (2 * p + 1) * 128 + half * 64 : (2 * p + 1) * 128 + half * 64 + 64] = (
                W_qkv[r0 + DK : r0 + 2 * DK].T
            )
            bqk[half * 64 : half * 64 + 64, 2 * p + 1] = b_qkv[r0 + DK : r0 + 2 * DK]
            wo[p * 128 + half * 64 : p * 128 + half * 64 + 64, :] = W_out[
                :, h * DK : (h + 1) * DK
            ].T
    for hh, h in enumerate(heads):
        r0 = h * 3 * DK + 2 * DK
        wv[:, hh * DK : (hh + 1) * DK] = W_qkv[r0 : r0 + DK].T
        bv[0, hh * DK : (hh + 1) * DK] = b_qkv[r0 : r0 + DK]
    return {
        "xT": xT.astype(BF16),
        "wqkT": wqk.astype(BF16),
        "wvT": wv.astype(BF16),
        "bqk": bqk.astype(np.float32),
        "bv": bv.astype(BF16),
        "woT": wo.astype(BF16),
        "trisq": make_tri_square().astype(BF16),
    }


_NC_CACHE = {}


def kernel(x, W_qkv, b_qkv, W_out, b_out, _trace=False):
    x = np.asarray(x, dtype=np.float32)
    W_qkv = np.asarray(W_qkv, dtype=np.float32)
    b_qkv = np.asarray(b_qkv, dtype=np.float32)
    W_out = np.asarray(W_out, dtype=np.float32)
    b_out = np.asarray(b_out, dtype=np.float32)

    from concourse.bass_utils import run_bass_kernel_spmd

    key = ("full", C, T, 4)
    if key not in _NC_CACHE:
        _NC_CACHE[key] = build_program(C, T, n_pairs=4, num_devices=1)
    nc = _NC_CACHE[key]

    in_maps = []
    for core in range(NCORES):
        b, hg = divmod(core, 2)
        heads = list(range(hg * 8, hg * 8 + 8))
        in_maps.append(make_core_inputs(x[b], W_qkv, b_qkv, W_out, heads))

    res = run_bass_kernel_spmd(nc, in_maps, list(range(NCORES)), trace=_trace)
    kernel._last_results = res

    out = np.broadcast_to(b_out, (B, T, C)).astype(np.float32).copy()
    for core in range(NCORES):
        b = core // 2
        out[b] += res.results[core]["out"].astype(np.float32)
    return out



# revision 3
# speedup vs baseline: 1.0488x; 1.0488x over previous
"""Causal self-attention Trainium2 kernel (B=4, T=2048, D=1024, H=16).

Sharding: 8 cores = 4 batches x 2 head-groups (8 heads each). Each core
computes its batch's qkv projection restricted to its 8 heads, causal
attention for those heads, and a partial out-projection over its 512 ctx
channels. Host sums the two partials per batch and adds b_out.

v2 restructure (from trace analysis of the 276us baseline):
  - Matmul slices cost ~170ns latency + F*0.42ns stream (F = moving free
    dim); K and M are free. The baseline's ctx matmuls used F=512 (t) with
    M=128 (64 d + 64 denominator copies), 139k columns. Flipped here to
    out[t=128, d+den=65]: lhsT = the exp'd attn tile (s x t slice, FWL
    weight loads), rhs = v_ext65 [v(64) | ones(1)], F=65 -> 71k columns.
    The denominator rides as the 65th column; normalize is a [128,1]
    per-partition reciprocal + scalar_tensor_tensor on the DVE; a per-tt
    DMA xbar transpose rebuilds the [d, t] ctxT layout the out-projection
    consumes (DMA engines are idle mid-kernel).
  - tt-major chains: per (pair, t-block) the s-chain replays already-exp'd
    attn tiles from SBUF, so most ctx matmuls do not wait on ScalarE.
  - ScalarE was 92% busy (exp 158us is irreducible; softmax needs it):
    the out-projection PSUM->SBUF copies moved from ScalarE Identity
    activations to DVE tensor_copy.
  - The prologue only computes pair-0's qk + v(i=0) before attention
    starts (exp work begins ~15us instead of 47us); the remaining qk
    projections, v(i+1), and out chunks ride as fillers inside the attn
    pair emission, sized per round to keep the PE busy while Scalar paces.
  - fp8 was tried previously and rejected: e4m3 noise (~2.6%/element)
    passes ~1:1 into the output (random-sign dot products), measured 4.6%
    rel err vs the 2e-2 gate.
"""

import math

import numpy as np
import ml_dtypes

B, T, C = 4, 2048, 1024
H, DK = 16, 64
NCORES = 8
TS = 128  # s-tile / t-block (partition granularity)
TSL = 512  # t free-dim tile of the scores (one 2-bank PSUM tile per pair-block)
BF16 = ml_dtypes.bfloat16
VE = DK + 1  # per-head ctx columns: 64 v channels + 1 denominator


def build_program(C_sz=C, T_sz=T, n_pairs=4, num_devices=1):
    import concourse.mybir as mybir
    from concourse import bacc
    from concourse.tile import TileContext

    dt = mybir.dt
    f32 = dt.float32
    bf16 = dt.bfloat16
    AF = mybir.ActivationFunctionType

    n_ct = C_sz // 128  # contraction tiles for projections
    n_qk = 2 * n_pairs  # qk o-tiles (128 channels each)
    VW = n_pairs * 2 * DK  # v channels (natural order)
    n_tt = T_sz // TS
    n_it = T_sz // TSL
    JPI = TSL // TS  # s-tiles per i-tile (4)
    OW = min(TSL, C_sz)  # output column tile width
    n_oh = C_sz // OW  # output column halves
    VEW = n_pairs * 2 * VE  # v_ext65 width

    nc = bacc.Bacc(
        "TRN2",
        target_bir_lowering=False,
        debug=False,
        num_devices=num_devices,
    )

    xT_d = nc.dram_tensor("xT", [C_sz, T_sz], bf16, kind="ExternalInput").ap()
    wqkA_d = nc.dram_tensor("wqkA", [C_sz, 2 * 128], bf16, kind="ExternalInput").ap()
    wqkB_d = nc.dram_tensor(
        "wqkB", [C_sz, (n_qk - 2) * 128], bf16, kind="ExternalInput"
    ).ap()
    wv_d = nc.dram_tensor("wvT", [C_sz, VW], bf16, kind="ExternalInput").ap()
    bqk_d = nc.dram_tensor("bqk", [128, n_qk], f32, kind="ExternalInput").ap()
    bv_d = nc.dram_tensor("bv", [1, VW], bf16, kind="ExternalInput").ap()
    wo_d = nc.dram_tensor("woT", [n_pairs * 128, C_sz], bf16, kind="ExternalInput").ap()
    tri_d = nc.dram_tensor("trisq", [128, 2 * TS], bf16, kind="ExternalInput").ap()
    out_d = nc.dram_tensor("out", [T_sz, C_sz], bf16, kind="ExternalOutput").ap()

    with TileContext(nc) as tc:
        with (
            tc.tile_pool(name="const", bufs=1) as const_pool,
            tc.tile_pool(name="big", bufs=1) as big_pool,
            tc.tile_pool(name="attn", bufs=22) as attn_pool,
            tc.tile_pool(name="nrm", bufs=6) as nrm_pool,
            tc.tile_pool(name="outsb", bufs=6) as outsb_pool,
            tc.tile_pool(name="sc", bufs=2, space="PSUM") as sc_ps,
            tc.tile_pool(name="mm", bufs=2, space="PSUM") as mm_ps,
            tc.tile_pool(name="cx", bufs=2, space="PSUM") as cx_ps,
        ):
            # ---- DMA order: the prologue (qk ot0,1 + v, ci-outer) consumes
            # xT[:,0:TSL], wqkA, wv per ci — those stream first so the first
            # matmuls start as soon as the queues warm up ----
            xT_sb = []
            wqk_sb = []
            wv_sb = []
            for ci in range(n_ct):
                t = big_pool.tile([128, T_sz], bf16, tag=f"xT{ci}", name=f"xT{ci}")
                nc.sync.dma_start(t[:, 0:TSL], xT_d[ci * 128 : (ci + 1) * 128, 0:TSL])
                xT_sb.append(t)
                t = big_pool.tile(
                    [128, n_qk * 128], bf16, tag=f"wqk{ci}", name=f"wqk{ci}"
                )
                nc.sync.dma_start(t[:, 0:256], wqkA_d[ci * 128 : (ci + 1) * 128, :])
                wqk_sb.append(t)
                t = big_pool.tile([128, VW], bf16, tag=f"wv{ci}", name=f"wv{ci}")
                nc.sync.dma_start(t[:], wv_d[ci * 128 : (ci + 1) * 128, :])
                wv_sb.append(t)
            bqk_sb = const_pool.tile([128, n_qk], f32, tag="bqk", name="bqk")
            nc.sync.dma_start(bqk_sb[:], bqk_d)
            tri_sb = const_pool.tile([128, 2 * TS], bf16, tag="tri", name="tri")
            nc.sync.dma_start(tri_sb[:], tri_d)
            bv_sb = const_pool.tile([1, VW], bf16, tag="bv", name="bv")
            nc.sync.dma_start(bv_sb[:], bv_d)
            bv_bc = const_pool.tile([128, VW], bf16, tag="bv_bc", name="bv_bc")
            nc.gpsimd.partition_broadcast(bv_bc[:], bv_sb[:])
            ones_bc = const_pool.tile([128, TSL], bf16, tag="ones_bc", name="ones_bc")
            nc.gpsimd.memset(ones_bc[:], 1.0)
            # remaining qk weights (ot 2..7), then the x columns needed from
            # round 1 on, then wo (first consumed by out(0) fillers in R1)
            for ci in range(n_ct):
                nc.sync.dma_start(
                    wqk_sb[ci][:, 256 : n_qk * 128],
                    wqkB_d[ci * 128 : (ci + 1) * 128, :],
                )
            for ci in range(n_ct):
                nc.sync.dma_start(
                    xT_sb[ci][:, TSL : 2 * TSL],
                    xT_d[ci * 128 : (ci + 1) * 128, TSL : 2 * TSL],
                )
            wo_sb = []
            for p in range(n_pairs):
                t = big_pool.tile([128, C_sz], bf16, tag=f"wo{p}", name=f"wo{p}")
                nc.sync.dma_start(t[:], wo_d[p * 128 : (p + 1) * 128, :])
                wo_sb.append(t)
            for ci in range(n_ct):
                nc.sync.dma_start(
                    xT_sb[ci][:, 2 * TSL : T_sz],
                    xT_d[ci * 128 : (ci + 1) * 128, 2 * TSL : T_sz],
                )

            qkT_sb = [
                big_pool.tile([128, T_sz], bf16, tag=f"qkT{ot}", name=f"qkT{ot}")
                for ot in range(n_qk)
            ]
            vext_sb = [
                big_pool.tile([128, VEW], bf16, tag=f"vext{tt}", name=f"vext{tt}")
                for tt in range(n_tt)
            ]
            for tt in range(n_tt):
                # one-time: the ones column of every per-head v_ext65 block
                vx3 = vext_sb[tt][:].rearrange("p (h e) -> p h e", e=VE)
                nc.gpsimd.memset(vx3[:, :, DK:VE], 1.0)
            ctxT_sb = [
                big_pool.tile([128, T_sz], bf16, tag=f"ctxT{p}", name=f"ctxT{p}")
                for p in range(n_pairs)
            ]

            def qk_copy(ot, i, ps):
                # (ps + bias) * 1.0 on the DVE: keeps the ScalarE free for
                # the softmax exps, which pace the whole kernel
                nc.vector.scalar_tensor_tensor(
                    qkT_sb[ot][:, i * TSL : (i + 1) * TSL],
                    ps[:],
                    bqk_sb[:, ot : ot + 1],
                    ones_bc[:],
                    op0=mybir.AluOpType.add,
                    op1=mybir.AluOpType.mult,
                )

            def qk_mms(ot, i, ps, c0, c1):
                for ci in range(c0, c1):
                    nc.tensor.matmul(
                        ps[:],
                        lhsT=wqk_sb[ci][:, ot * 128 : (ot + 1) * 128],
                        rhs=xT_sb[ci][:, i * TSL : (i + 1) * TSL],
                        start=(ci == 0),
                        stop=(ci == n_ct - 1),
                    )

            def qk_units(ot, i):
                # a qk projection split into two ~1.7us filler units;
                # the PSUM tile is allocated lazily at emission time
                box = {}

                def head():
                    box["ps"] = mm_ps.tile([128, TSL], f32, tag="mm", name="mm")
                    qk_mms(ot, i, box["ps"], 0, n_ct // 2)

                def tail():
                    qk_mms(ot, i, box["ps"], n_ct // 2, n_ct)
                    qk_copy(ot, i, box["ps"])

                return [head, tail]

            def v_mms(tt, ps_ap, c0, c1):
                for ci in range(c0, c1):
                    nc.tensor.matmul(
                        ps_ap,
                        lhsT=xT_sb[ci][:, tt * TS : (tt + 1) * TS],
                        rhs=wv_sb[ci][:],
                        start=(ci == 0),
                        stop=(ci == n_ct - 1),
                        skip_group_check=True,
                    )

            def v_finish(tt, ps_ap):
                # ps_ap: [128, VW] fp32 PSUM (v channels only; the ones
                # column of each head block is memset once in the prologue)
                vx3 = vext_sb[tt][:].rearrange("p (h e) -> p h e", e=VE)
                nc.vector.scalar_tensor_tensor(
                    vx3[:, :, 0:DK],
                    ps_ap.rearrange("p (h e) -> p h e", e=DK),
                    1.0,
                    bv_bc[:].rearrange("p (h e) -> p h e", e=DK),
                    op0=mybir.AluOpType.mult,
                    op1=mybir.AluOpType.add,
                )

            def v_units(tt):
                box = {}

                def head():
                    box["ps"] = mm_ps.tile([128, VW], f32, tag="mm", name="mm")
                    v_mms(tt, box["ps"][:], 0, n_ct // 2)

                def tail():
                    v_mms(tt, box["ps"][:], n_ct // 2, n_ct)
                    v_finish(tt, box["ps"][:])

                return [head, tail]

            def out_chunk(i, c):
                tt, oh = JPI * i + c // n_oh, c % n_oh
                ps = mm_ps.tile([128, OW], f32, tag="mm", name="mm")
                for p in range(n_pairs):
                    nc.tensor.matmul(
                        ps[:],
                        lhsT=ctxT_sb[p][:, tt * TS : (tt + 1) * TS],
                        rhs=wo_sb[p][:, oh * OW : (oh + 1) * OW],
                        start=(p == 0),
                        stop=(p == n_pairs - 1),
                    )
                ob = outsb_pool.tile([128, OW], bf16, tag="outsb", name="outsb")
                nc.vector.tensor_copy(ob[:], ps[:])
                nc.sync.dma_start(
                    out_d[tt * TS : (tt + 1) * TS, oh * OW : (oh + 1) * OW],
                    ob[:],
                )

            def attn_pair(p, i, fillers=None, post_tt=None):
                """tt-major: for each local t-block lt, one PSUM chain over
                s-blocks j=0..4i+lt with F=65 (64 v channels + denominator).
                Scores/exp run 2 blocks ahead; phase-B chains replay SBUF
                attn tiles so they do not wait on ScalarE. Fillers: one per
                leg-1 j-step, two per phase-B t-block (JPI*i + 7 total)."""
                qt, kt = qkT_sb[2 * p], qkT_sb[2 * p + 1]
                nj = JPI * (i + 1)
                tri3 = tri_sb[:].rearrange("p (c w) -> p c w", c=2)
                a_tiles = {}

                def take_filler():
                    if fillers:
                        fillers.pop(0)()

                def scores_block(j):
                    diag = j >= JPI * i
                    pi = j - JPI * i if diag else 0
                    t0 = pi * TS  # first causally-live t column in this block
                    ps = sc_ps.tile([128, 2 * TSL], f32, tag="sc", name="sc")
                    nc.tensor.matmul(
                        ps[:, t0:TSL],
                        lhsT=kt[0:64, j * TS : (j + 1) * TS],
                        rhs=qt[0:64, i * TSL + t0 : (i + 1) * TSL],
                        start=True,
                        stop=True,
                        skip_group_check=True,
                    )
                    nc.tensor.matmul(
                        ps[:, TSL + t0 : 2 * TSL],
                        lhsT=kt[64:128, j * TS : (j + 1) * TS],
                        rhs=qt[64:128, i * TSL + t0 : (i + 1) * TSL],
                        start=True,
                        stop=True,
                        skip_group_check=True,
                    )
                    a = attn_pool.tile([128, 2 * TSL], bf16, tag="attn", name="attn")
                    a3 = a[:].rearrange("p (c w) -> p c w", c=2)
                    ps3 = ps[:].rearrange("p (c w) -> p c w", c=2)
                    nc.scalar.activation(a3[:, :, t0:TSL], ps3[:, :, t0:TSL], AF.Exp)
                    if diag:
                        # zero the below-diagonal triangle of the 128x128
                        # square (exp of unmasked scores is finite garbage)
                        nc.vector.tensor_mul(
                            a3[:, :, t0 : t0 + TS], a3[:, :, t0 : t0 + TS], tri3
                        )
                    return a, t0

                pend = [scores_block(j) for j in range(min(2, nj))]

                def pop_scores(j):
                    a_tiles[j] = pend.pop(0)
                    if j + 2 < nj:
                        pend.append(scores_block(j + 2))

                def ctx_mms(cx, lt, j, j_last):
                    # PSUM zero-region semantics: start=True arms
                    # overwrite-on-next-write for the WHOLE 2KB bank, so the
                    # two head chains sharing this bank must have exactly ONE
                    # start event (head A j=0) — head B's j=0 bytes are still
                    # armed from it and overwrite correctly.
                    a, _ = a_tiles[j]
                    for h in (0, 1):
                        nc.tensor.matmul(
                            cx[:, h * VE : (h + 1) * VE],
                            lhsT=a[:, h * TSL + lt * TS : h * TSL + (lt + 1) * TS],
                            rhs=vext_sb[j][:, (2 * p + h) * VE : (2 * p + h + 1) * VE],
                            start=(j == 0 and h == 0),
                            stop=(j == j_last and h == 1),
                            skip_group_check=True,
                        )

                def normalize(lt, cx):
                    # cx: [128 t, 2*VE]; col 64/129 hold the denominators
                    cx3 = cx[:].rearrange("q (h e) -> q h e", e=VE)
                    den = nrm_pool.tile([128, 2], f32, tag="den", name="den")
                    nc.vector.tensor_copy(
                        den[:].rearrange("q (h e) -> q h e", e=1),
                        cx3[:, :, DK : DK + 1],
                    )
                    rec = nrm_pool.tile([128, 2], f32, tag="rec", name="rec")
                    nc.vector.reciprocal_approx_fast(rec[:], den[:])
                    cf = nrm_pool.tile([128, 2 * DK], bf16, tag="cf", name="cf")
                    for h in (0, 1):
                        nc.vector.scalar_tensor_tensor(
                            cf[:, h * DK : (h + 1) * DK],
                            cx[:, h * VE : h * VE + DK],
                            rec[:, h : h + 1],
                            ones_bc[:, 0:DK],
                            op0=mybir.AluOpType.mult,
                            op1=mybir.AluOpType.mult,
                        )
                    gt = JPI * i + lt
                    # xbar transpose [t, d] -> [d, t] into the out-proj's
                    # stationary ctxT layout (idle DMA engines mid-kernel)
                    nc.sync.dma_start(
                        ctxT_sb[p][:, gt * TS : (gt + 1) * TS],
                        cf[:],
                        transpose=True,
                    )

                # leg 1: t-block lt=0, j = 0..JPI*i (the bulk of the exps)
                j_last = JPI * i
                cx = cx_ps.tile([128, 2 * VE], f32, tag="cx", name="cx")
                for j in range(j_last + 1):
                    pop_scores(j)
                    ctx_mms(cx, 0, j, j_last)
                    take_filler()
                normalize(0, cx)
                if post_tt:
                    post_tt(0)
                # phase B: lt = 1..3 — one new diag scores block each, then
                # a replay chain over all earlier s-blocks
                for lt in range(1, JPI):
                    j_last = JPI * i + lt
                    pop_scores(j_last)
                    take_filler()
                    cx = cx_ps.tile([128, 2 * VE], f32, tag="cx", name="cx")
                    for j in range(j_last + 1):
                        ctx_mms(cx, lt, j, j_last)
                        if j == j_last // 2:
                            take_filler()
                    normalize(lt, cx)
                    if post_tt:
                        post_tt(lt)

            def run_pairs(pairs, fillers, post_tts=None):
                """Emit attn pairs with the filler units spread evenly over
                their filler slots (order-preserving; pads with no-ops)."""
                nslots = sum(JPI * ii + 7 for _, ii in pairs)
                k = len(fillers)
                spaced = []
                for bi in range(nslots):
                    take = (bi * k) // nslots != ((bi + 1) * k) // nslots
                    spaced.append(fillers[(bi * k) // nslots] if take else None)
                for p, ii in pairs:
                    ns = JPI * ii + 7
                    attn_pair(
                        p,
                        ii,
                        fillers=[(u or (lambda: None)) for u in spaced[:ns]],
                        post_tt=(post_tts or {}).get(p),
                    )
                    spaced = spaced[ns:]

            # ---- prologue: qk(ot0,1, i=0) + v(i=0), ci-outer so the first
            # matmuls only wait on the first DMA tiles ----
            pss = [mm_ps.tile([128, TSL], f32, tag="mm", name="mm") for _ in range(2)]
            vsc = [
                sc_ps.tile([128, 2 * TSL], f32, tag="sc", name="sc") for _ in range(2)
            ]
            vap = [
                vsc[tt // 2][:, (tt % 2) * TSL : (tt % 2) * TSL + VW]
                for tt in range(JPI)
            ]
            for ci in range(n_ct):
                for oi in range(2):
                    nc.tensor.matmul(
                        pss[oi][:],
                        lhsT=wqk_sb[ci][:, oi * 128 : (oi + 1) * 128],
                        rhs=xT_sb[ci][:, 0:TSL],
                        start=(ci == 0),
                        stop=(ci == n_ct - 1),
                    )
                for tt in range(JPI):
                    v_mms(tt, vap[tt], ci, ci + 1)
            for oi in range(2):
                qk_copy(oi, 0, pss[oi])
            for tt in range(JPI):
                v_finish(tt, vap[tt])

            # ---- rounds r = 0..3: attn pairs (p, r) with the next round's
            # projections and the previous round's out chunks as fillers ----
            for r in range(n_it):
                if r > 0:
                    for ot in (0, 1):
                        for u in qk_units(ot, r):
                            u()
                fillers = []
                for ot in range(2, n_qk):
                    fillers.extend(qk_units(ot, r))
                if r + 1 < n_it:
                    for tt in range(JPI * (r + 1), JPI * (r + 2)):
                        fillers.extend(v_units(tt))
                if r >= 1:
                    fillers.extend(
                        (lambda cc: lambda: out_chunk(r - 1, cc))(c)
                        for c in range(2 * JPI)
                    )
                post_tts = None
                if r == n_it - 1:
                    # emit out(r) chunks for t-block gt one lt later, so the
                    # chunk's p=3 matmul never head-blocks the PE queue on
                    # the just-enqueued DMA transpose
                    def final_post(lt):
                        if lt >= 1:
                            tt = JPI * r + lt - 1
                            for oh in range(n_oh):
                                out_chunk(r, (tt - JPI * r) * n_oh + oh)

                    post_tts = {n_pairs - 1: final_post}
                run_pairs([(p, r) for p in range(n_pairs)], fillers, post_tts)

            # final: the last t-block's out chunks
            for oh in range(n_oh):
                out_chunk(n_it - 1, (JPI - 1) * n_oh + oh)

    nc.compile()
    return nc


def make_tri_square(ts=TS):
    """[128, 2*ts] {0,1} keep-mask, duplicated per head: cell (s, t) = 0 iff
    s > t (strictly below the diagonal of the 128x128 square)."""
    s = np.arange(128)[:, None]
    t = np.arange(ts)[None, :]
    one = np.where(s > t, 0.0, 1.0).astype(np.float32)
    return np.concatenate([one, one], axis=1)


def make_core_inputs(x_b, W_qkv, b_qkv, W_out, heads, C_sz=C, T_sz=T):
    """Build the per-core input map (numpy, host-side)."""
    n_pairs = len(heads) // 2
    n_qk = 2 * n_pairs
    VW = len(heads) * DK
    xT = np.ascontiguousarray(x_b.T).astype(np.float32)
    wqk = np.empty((C_sz, n_qk * 128), np.float32)
    bqk = np.empty((128, n_qk), np.float32)
    wv = np.empty((C_sz, VW), np.float32)
    bv = np.empty((1, VW), np.float32)
    wo = np.empty((n_pairs * 128, C_sz), np.float32)
    for p in range(n_pairs):
        hA, hB = heads[2 * p], heads[2 * p + 1]
        # q tile (scaled by 1/sqrt(dk)=1/8), k tile
        for half, h in ((0, hA), (1, hB)):
            r0 = h * 3 * DK
            wqk[:, 2 * p * 128 + half * 64 : 2 * p * 128 + half * 64 + 64] = (
                W_qkv[r0 : r0 + DK].T / math.sqrt(DK)
            )
            bqk[half * 64 : half * 64 + 64, 2 * p] = b_qkv[r0 : r0 + DK] / math.sqrt(DK)
            wqk[:, (2 * p + 1) * 128 + half * 64 : (2 * p + 1) * 128 + half * 64 + 64] = (
                W_qkv[r0 + DK : r0 + 2 * DK].T
            )
            bqk[half * 64 : half * 64 + 64, 2 * p + 1] = b_qkv[r0 + DK : r0 + 2 * DK]
            wo[p * 128 + half * 64 : p * 128 + half * 64 + 64, :] = W_out[
                :, h * DK : (h + 1) * DK
            ].T
    for hh, h in enumerate(heads):
        r0 = h * 3 * DK + 2 * DK
        wv[:, hh * DK : (hh + 1) * DK] = W_qkv[r0 : r0 + DK].T
        bv[0, hh * DK : (hh + 1) * DK] = b_qkv[r0 : r0 + DK]
    return {
        "xT": xT.astype(BF16),
        "wqkA": np.ascontiguousarray(wqk[:, 0:256]).astype(BF16),
        "wqkB": np.ascontiguousarray(wqk[:, 256:]).astype(BF16),
        "wvT": wv.astype(BF16),
        "bqk": bqk.astype(np.float32),
        "bv": bv.astype(BF16),
        "woT": wo.astype(BF16),
        "trisq": make_tri_square().astype(BF16),
    }


_NC_CACHE = {}


def kernel(x, W_qkv, b_qkv, W_out, b_out, _trace=False):
    x = np.asarray(x, dtype=np.float32)
    W_qkv = np.asarray(W_qkv, dtype=np.float32)
    b_qkv = np.asarray(b_qkv, dtype=np.float32)
    W_out = np.asarray(W_out, dtype=np.float32)
    b_out = np.asarray(b_out, dtype=np.float32)

    from concourse.bass_utils import run_bass_kernel_spmd

    key = ("full", C, T, 4)
    if key not in _NC_CACHE:
        _NC_CACHE[key] = build_program(C, T, n_pairs=4, num_devices=1)
    nc = _NC_CACHE[key]

    in_maps = []
    for core in range(NCORES):
        b, hg = divmod(core, 2)
        heads = list(range(hg * 8, hg * 8 + 8))
        in_maps.append(make_core_inputs(x[b], W_qkv, b_qkv, W_out, heads))

    res = run_bass_kernel_spmd(nc, in_maps, list(range(NCORES)), trace=_trace)
    kernel._last_results = res

    out = np.broadcast_to(b_out, (B, T, C)).astype(np.float32).copy()
    for core in range(NCORES):
        b = core // 2
        out[b] += res.results[core]["out"].astype(np.float32)
    return out


# revision 10
# speedup vs baseline: 1.0899x; 1.0392x over previous
"""Causal self-attention Trainium2 kernel (B=4, T=2048, D=1024, H=16).

Sharding: 8 cores = 4 batches x 2 head-groups (8 heads each). Each core
computes its batch's qkv projection restricted to its 8 heads, causal
attention for those heads, and a partial out-projection over its 512 ctx
channels. Host sums the two partials per batch and adds b_out.

v2 restructure (from trace analysis of the 276us baseline):
  - Matmul slices cost ~170ns latency + F*0.42ns stream (F = moving free
    dim); K and M are free. The baseline's ctx matmuls used F=512 (t) with
    M=128 (64 d + 64 denominator copies), 139k columns. Flipped here to
    out[t=128, d+den=65]: lhsT = the exp'd attn tile (s x t slice, FWL
    weight loads), rhs = v_ext65 [v(64) | ones(1)], F=65 -> 71k columns.
    The denominator rides as the 65th column; normalize is a [128,1]
    per-partition reciprocal + scalar_tensor_tensor on the DVE; a per-tt
    DMA xbar transpose rebuilds the [d, t] ctxT layout the out-projection
    consumes (DMA engines are idle mid-kernel).
  - tt-major chains: per (pair, t-block) the s-chain replays already-exp'd
    attn tiles from SBUF, so most ctx matmuls do not wait on ScalarE.
  - ScalarE was 92% busy (exp 158us is irreducible; softmax needs it):
    the out-projection PSUM->SBUF copies moved from ScalarE Identity
    activations to DVE tensor_copy.
  - The prologue only computes pair-0's qk + v(i=0) before attention
    starts (exp work begins ~15us instead of 47us); the remaining qk
    projections, v(i+1), and out chunks ride as fillers inside the attn
    pair emission, sized per round to keep the PE busy while Scalar paces.
  - fp8 was tried previously and rejected: e4m3 noise (~2.6%/element)
    passes ~1:1 into the output (random-sign dot products), measured 4.6%
    rel err vs the 2e-2 gate.
"""

import math

import numpy as np
import ml_dtypes

B, T, C = 4, 2048, 1024
H, DK = 16, 64
NCORES = 8
TS = 128  # s-tile / t-block (partition granularity)
TSL = 512  # t free-dim tile of the scores (one 2-bank PSUM tile per pair-block)
BF16 = ml_dtypes.bfloat16
VE = DK + 1  # per-head ctx columns: 64 v channels + 1 denominator


def build_program(C_sz=C, T_sz=T, n_pairs=4, num_devices=1):
    import concourse.mybir as mybir
    from concourse import bacc
    from concourse.tile import TileContext

    dt = mybir.dt
    f32 = dt.float32
    bf16 = dt.bfloat16
    AF = mybir.ActivationFunctionType

    n_ct = C_sz // 128  # contraction tiles for projections
    n_qk = 2 * n_pairs  # qk o-tiles (128 channels each)
    VW = n_pairs * 2 * DK  # v channels (natural order)
    n_tt = T_sz // TS
    n_it = T_sz // TSL
    JPI = TSL // TS  # s-tiles per i-tile (4)
    OW = min(TSL, C_sz)  # output column tile width
    n_oh = C_sz // OW  # output column halves
    VEW = n_pairs * 2 * VE  # v_ext65 width

    nc = bacc.Bacc(
        "TRN2",
        target_bir_lowering=False,
        debug=False,
        num_devices=num_devices,
    )

    xT_d = nc.dram_tensor("xT", [C_sz, T_sz], bf16, kind="ExternalInput").ap()
    wqkA_d = nc.dram_tensor("wqkA", [C_sz, 2 * 128], bf16, kind="ExternalInput").ap()
    wqkB_d = nc.dram_tensor(
        "wqkB", [C_sz, (n_qk - 2) * 128], bf16, kind="ExternalInput"
    ).ap()
    wv_d = nc.dram_tensor("wvT", [C_sz, VW], bf16, kind="ExternalInput").ap()
    bqk_d = nc.dram_tensor("bqk", [128, n_qk], f32, kind="ExternalInput").ap()
    bv_d = nc.dram_tensor("bv", [1, VW], bf16, kind="ExternalInput").ap()
    wo_d = nc.dram_tensor("woT", [n_pairs * 128, C_sz], bf16, kind="ExternalInput").ap()
    tri_d = nc.dram_tensor("trisq", [128, 2 * TS], bf16, kind="ExternalInput").ap()
    id_d = nc.dram_tensor("ident", [128, 128], bf16, kind="ExternalInput").ap()
    out_d = nc.dram_tensor("out", [T_sz, C_sz], bf16, kind="ExternalOutput").ap()

    with TileContext(nc) as tc:
        with (
            tc.tile_pool(name="const", bufs=1) as const_pool,
            tc.tile_pool(name="big", bufs=1) as big_pool,
            tc.tile_pool(name="attn", bufs=22) as attn_pool,
            tc.tile_pool(name="nrm", bufs=6) as nrm_pool,
            tc.tile_pool(name="outsb", bufs=6) as outsb_pool,
            tc.tile_pool(name="sc", bufs=2, space="PSUM") as sc_ps,
            tc.tile_pool(name="mm", bufs=2, space="PSUM") as mm_ps,
            tc.tile_pool(name="cx", bufs=2, space="PSUM") as cx_ps,
        ):
            # ---- DMA order: the prologue (qk ot0,1 + v, ci-outer) consumes
            # xT[:,0:TSL], wqkA, wv per ci — those stream first so the first
            # matmuls start as soon as the queues warm up ----
            xT_sb = []
            wqk_sb = []
            wv_sb = []
            for ci in range(n_ct):
                t = big_pool.tile([128, T_sz], bf16, tag=f"xT{ci}", name=f"xT{ci}")
                nc.sync.dma_start(t[:, 0:TSL], xT_d[ci * 128 : (ci + 1) * 128, 0:TSL])
                xT_sb.append(t)
                t = big_pool.tile(
                    [128, n_qk * 128], bf16, tag=f"wqk{ci}", name=f"wqk{ci}"
                )
                nc.sync.dma_start(t[:, 0:256], wqkA_d[ci * 128 : (ci + 1) * 128, :])
                wqk_sb.append(t)
                t = big_pool.tile([128, VW], bf16, tag=f"wv{ci}", name=f"wv{ci}")
                nc.sync.dma_start(t[:], wv_d[ci * 128 : (ci + 1) * 128, :])
                wv_sb.append(t)
            bqk_sb = const_pool.tile([128, n_qk], f32, tag="bqk", name="bqk")
            nc.sync.dma_start(bqk_sb[:], bqk_d)
            tri_sb = const_pool.tile([128, 2 * TS], bf16, tag="tri", name="tri")
            nc.sync.dma_start(tri_sb[:], tri_d)
            bv_sb = const_pool.tile([1, VW], bf16, tag="bv", name="bv")
            nc.sync.dma_start(bv_sb[:], bv_d)
            id_sb = const_pool.tile([128, 128], bf16, tag="ident", name="ident")
            nc.sync.dma_start(id_sb[:], id_d)
            bv_bc = const_pool.tile([128, VW], bf16, tag="bv_bc", name="bv_bc")
            nc.gpsimd.partition_broadcast(bv_bc[:], bv_sb[:])
            ones_bc = const_pool.tile([128, TSL], bf16, tag="ones_bc", name="ones_bc")
            nc.gpsimd.memset(ones_bc[:], 1.0)
            # remaining qk weights (ot 2..7), then the x columns needed from
            # round 1 on, then wo (first consumed by out(0) fillers in R1)
            for ci in range(n_ct):
                nc.sync.dma_start(
                    wqk_sb[ci][:, 256 : n_qk * 128],
                    wqkB_d[ci * 128 : (ci + 1) * 128, :],
                )
            for ci in range(n_ct):
                nc.sync.dma_start(
                    xT_sb[ci][:, TSL : 2 * TSL],
                    xT_d[ci * 128 : (ci + 1) * 128, TSL : 2 * TSL],
                )
            for ci in range(n_ct):
                nc.sync.dma_start(
                    xT_sb[ci][:, 2 * TSL : 3 * TSL],
                    xT_d[ci * 128 : (ci + 1) * 128, 2 * TSL : 3 * TSL],
                )
            wo_sb = []
            for p in range(n_pairs):
                t = big_pool.tile([128, C_sz], bf16, tag=f"wo{p}", name=f"wo{p}")
                nc.sync.dma_start(t[:], wo_d[p * 128 : (p + 1) * 128, :])
                wo_sb.append(t)
            for ci in range(n_ct):
                nc.sync.dma_start(
                    xT_sb[ci][:, 3 * TSL : T_sz],
                    xT_d[ci * 128 : (ci + 1) * 128, 3 * TSL : T_sz],
                )

            qkT_sb = [
                big_pool.tile([128, T_sz], bf16, tag=f"qkT{ot}", name=f"qkT{ot}")
                for ot in range(n_qk)
            ]
            vext_sb = [
                big_pool.tile([128, VEW], bf16, tag=f"vext{tt}", name=f"vext{tt}")
                for tt in range(n_tt)
            ]
            for tt in range(n_tt):
                # one-time: the ones column of every per-head v_ext65 block
                vx3 = vext_sb[tt][:].rearrange("p (h e) -> p h e", e=VE)
                nc.gpsimd.memset(vx3[:, :, DK:VE], 1.0)
            ctxT_sb = [
                big_pool.tile([128, T_sz], bf16, tag=f"ctxT{p}", name=f"ctxT{p}")
                for p in range(n_pairs)
            ]

            def qk_copy(ot, i, ps):
                # (ps + bias) * 1.0 on the DVE: keeps the ScalarE free for
                # the softmax exps, which pace the whole kernel
                nc.vector.scalar_tensor_tensor(
                    qkT_sb[ot][:, i * TSL : (i + 1) * TSL],
                    ps[:],
                    bqk_sb[:, ot : ot + 1],
                    ones_bc[:],
                    op0=mybir.AluOpType.add,
                    op1=mybir.AluOpType.mult,
                )

            def qk_mms(ot, i, ps, c0, c1):
                for ci in range(c0, c1):
                    nc.tensor.matmul(
                        ps[:],
                        lhsT=wqk_sb[ci][:, ot * 128 : (ot + 1) * 128],
                        rhs=xT_sb[ci][:, i * TSL : (i + 1) * TSL],
                        start=(ci == 0),
                        stop=(ci == n_ct - 1),
                    )

            def qk_units(ot, i):
                # a qk projection split into two ~1.7us filler units;
                # the PSUM tile is allocated lazily at emission time
                box = {}

                def head():
                    box["ps"] = mm_ps.tile([128, TSL], f32, tag="mm", name="mm")
                    qk_mms(ot, i, box["ps"], 0, n_ct // 2)

                def tail():
                    qk_mms(ot, i, box["ps"], n_ct // 2, n_ct)
                    qk_copy(ot, i, box["ps"])

                return [head, tail]

            def v_mms(tt, ps_ap, c0, c1):
                for ci in range(c0, c1):
                    nc.tensor.matmul(
                        ps_ap,
                        lhsT=xT_sb[ci][:, tt * TS : (tt + 1) * TS],
                        rhs=wv_sb[ci][:],
                        start=(ci == 0),
                        stop=(ci == n_ct - 1),
                        skip_group_check=True,
                    )

            def v_finish(tt, ps_ap):
                # ps_ap: [128, VW] fp32 PSUM (v channels only; the ones
                # column of each head block is memset once in the prologue)
                vx3 = vext_sb[tt][:].rearrange("p (h e) -> p h e", e=VE)
                nc.vector.scalar_tensor_tensor(
                    vx3[:, :, 0:DK],
                    ps_ap.rearrange("p (h e) -> p h e", e=DK),
                    1.0,
                    bv_bc[:].rearrange("p (h e) -> p h e", e=DK),
                    op0=mybir.AluOpType.mult,
                    op1=mybir.AluOpType.add,
                )

            def v_units(tt):
                box = {}

                def head():
                    box["ps"] = mm_ps.tile([128, VW], f32, tag="mm", name="mm")
                    v_mms(tt, box["ps"][:], 0, n_ct // 2)

                def tail():
                    v_mms(tt, box["ps"][:], n_ct // 2, n_ct)
                    v_finish(tt, box["ps"][:])

                return [head, tail]

            def out_chunk(i, c):
                tt, oh = JPI * i + c // n_oh, c % n_oh
                ps = mm_ps.tile([128, OW], f32, tag="mm", name="mm")
                for p in range(n_pairs):
                    nc.tensor.matmul(
                        ps[:],
                        lhsT=ctxT_sb[p][:, tt * TS : (tt + 1) * TS],
                        rhs=wo_sb[p][:, oh * OW : (oh + 1) * OW],
                        start=(p == 0),
                        stop=(p == n_pairs - 1),
                    )
                ob = outsb_pool.tile([128, OW], bf16, tag="outsb", name="outsb")
                nc.vector.tensor_copy(ob[:], ps[:])
                nc.sync.dma_start(
                    out_d[tt * TS : (tt + 1) * TS, oh * OW : (oh + 1) * OW],
                    ob[:],
                )

            def attn_pair(p, i, fillers=None, post_tt=None):
                """tt-major: for each local t-block lt, one PSUM chain over
                s-blocks j=0..4i+lt with F=65 (64 v channels + denominator).
                Scores/exp run 2 blocks ahead; phase-B chains replay SBUF
                attn tiles so they do not wait on ScalarE. Fillers: one per
                leg-1 j-step, two per phase-B t-block (JPI*i + 7 total)."""
                qt, kt = qkT_sb[2 * p], qkT_sb[2 * p + 1]
                nj = JPI * (i + 1)
                tri3 = tri_sb[:].rearrange("p (c w) -> p c w", c=2)
                a_tiles = {}

                def take_filler():
                    if fillers:
                        fillers.pop(0)()

                def scores_block(j):
                    diag = j >= JPI * i
                    pi = j - JPI * i if diag else 0
                    t0 = pi * TS  # first causally-live t column in this block
                    ps = sc_ps.tile([128, 2 * TSL], f32, tag="sc", name="sc")
                    nc.tensor.matmul(
                        ps[:, t0:TSL],
                        lhsT=kt[0:64, j * TS : (j + 1) * TS],
                        rhs=qt[0:64, i * TSL + t0 : (i + 1) * TSL],
                        start=True,
                        stop=True,
                        skip_group_check=True,
                    )
                    nc.tensor.matmul(
                        ps[:, TSL + t0 : 2 * TSL],
                        lhsT=kt[64:128, j * TS : (j + 1) * TS],
                        rhs=qt[64:128, i * TSL + t0 : (i + 1) * TSL],
                        start=True,
                        stop=True,
                        skip_group_check=True,
                    )
                    a = attn_pool.tile([128, 2 * TSL], bf16, tag="attn", name="attn")
                    a3 = a[:].rearrange("p (c w) -> p c w", c=2)
                    ps3 = ps[:].rearrange("p (c w) -> p c w", c=2)
                    nc.scalar.activation(a3[:, :, t0:TSL], ps3[:, :, t0:TSL], AF.Exp)
                    if diag:
                        # zero the below-diagonal triangle of the 128x128
                        # square (exp of unmasked scores is finite garbage)
                        nc.vector.tensor_mul(
                            a3[:, :, t0 : t0 + TS], a3[:, :, t0 : t0 + TS], tri3
                        )
                    return a, t0

                pend = [scores_block(j) for j in range(min(2, nj))]

                def pop_scores(j):
                    a_tiles[j] = pend.pop(0)
                    if j + 2 < nj:
                        pend.append(scores_block(j + 2))

                def ctx_mms(cx, lt, j, j_last):
                    # PSUM zero-region semantics: start=True arms
                    # overwrite-on-next-write for the WHOLE 2KB bank, so the
                    # two head chains sharing this bank must have exactly ONE
                    # start event (head A j=0) — head B's j=0 bytes are still
                    # armed from it and overwrite correctly.
                    a, _ = a_tiles[j]
                    for h in (0, 1):
                        nc.tensor.matmul(
                            cx[:, h * VE : (h + 1) * VE],
                            lhsT=a[:, h * TSL + lt * TS : h * TSL + (lt + 1) * TS],
                            rhs=vext_sb[j][:, (2 * p + h) * VE : (2 * p + h + 1) * VE],
                            start=(j == 0 and h == 0),
                            stop=(j == j_last and h == 1),
                            skip_group_check=True,
                        )

                pend_tr = []

                def normalize(lt, cx):
                    # cx: [128 t, 2*VE]; col 64/129 hold the denominators
                    cx3 = cx[:].rearrange("q (h e) -> q h e", e=VE)
                    den = nrm_pool.tile([128, 2], f32, tag="den", name="den")
                    nc.vector.tensor_copy(
                        den[:].rearrange("q (h e) -> q h e", e=1),
                        cx3[:, :, DK : DK + 1],
                    )
                    rec = nrm_pool.tile([128, 2], f32, tag="rec", name="rec")
                    nc.vector.reciprocal_approx_fast(rec[:], den[:])
                    cf = nrm_pool.tile([128, 2 * DK], bf16, tag="cf", name="cf")
                    for h in (0, 1):
                        nc.vector.scalar_tensor_tensor(
                            cf[:, h * DK : (h + 1) * DK],
                            cx[:, h * VE : h * VE + DK],
                            rec[:, h : h + 1],
                            ones_bc[:, 0:DK],
                            op0=mybir.AluOpType.mult,
                            op1=mybir.AluOpType.mult,
                        )
                    pend_tr.append((lt, cf))

                def flush_tr():
                    # PE-transpose [t, d] -> [d, t] into the out-proj's
                    # stationary ctxT layout. Deferred to mid-next-chain so
                    # the in-order PE queue never head-blocks on the DVE
                    # normalize that produces cf.
                    if not pend_tr:
                        return
                    lt, cf = pend_tr.pop(0)
                    gt = JPI * i + lt
                    tp = cx_ps.tile([128, TS], bf16, tag="cx", name="tp")
                    nc.tensor.transpose(tp[:], cf[:], id_sb[:])
                    nc.vector.tensor_copy(ctxT_sb[p][:, gt * TS : (gt + 1) * TS], tp[:])

                # leg 1: t-block lt=0, j = 0..JPI*i (the bulk of the exps)
                j_last = JPI * i
                cx = cx_ps.tile([128, 2 * VE], f32, tag="cx", name="cx")
                for j in range(j_last + 1):
                    pop_scores(j)
                    ctx_mms(cx, 0, j, j_last)
                    take_filler()
                normalize(0, cx)
                if post_tt:
                    post_tt(0)
                # phase B: lt = 1..3 — one new diag scores block each, then
                # a replay chain over all earlier s-blocks
                for lt in range(1, JPI):
                    j_last = JPI * i + lt
                    pop_scores(j_last)
                    take_filler()
                    cx = cx_ps.tile([128, 2 * VE], f32, tag="cx", name="cx")
                    for j in range(j_last + 1):
                        ctx_mms(cx, lt, j, j_last)
                        if j == j_last // 2:
                            take_filler()
                            flush_tr()
                    normalize(lt, cx)
                    if post_tt:
                        post_tt(lt)
                flush_tr()

            def run_pairs(pairs, fillers, post_tts=None):
                """Emit attn pairs with the filler units spread evenly over
                their filler slots (order-preserving; pads with no-ops)."""
                nslots = sum(JPI * ii + 7 for _, ii in pairs)
                k = len(fillers)
                spaced = []
                for bi in range(nslots):
                    take = (bi * k) // nslots != ((bi + 1) * k) // nslots
                    spaced.append(fillers[(bi * k) // nslots] if take else None)
                for p, ii in pairs:
                    ns = JPI * ii + 7
                    attn_pair(
                        p,
                        ii,
                        fillers=[(u or (lambda: None)) for u in spaced[:ns]],
                        post_tt=(post_tts or {}).get(p),
                    )
                    spaced = spaced[ns:]

            # ---- prologue: qk(ot0,1, i=0) + v(i=0), ci-outer so the first
            # matmuls only wait on the first DMA tiles ----
            pss = [mm_ps.tile([128, TSL], f32, tag="mm", name="mm") for _ in range(2)]
            vsc = [
                sc_ps.tile([128, 2 * TSL], f32, tag="sc", name="sc") for _ in range(2)
            ]
            vap = [
                vsc[tt // 2][:, (tt % 2) * TSL : (tt % 2) * TSL + VW]
                for tt in range(JPI)
            ]
            for ci in range(n_ct):
                for oi in range(2):
                    nc.tensor.matmul(
                        pss[oi][:],
                        lhsT=wqk_sb[ci][:, oi * 128 : (oi + 1) * 128],
                        rhs=xT_sb[ci][:, 0:TSL],
                        start=(ci == 0),
                        stop=(ci == n_ct - 1),
                    )
                for tt in range(JPI):
                    v_mms(tt, vap[tt], ci, ci + 1)
            for oi in range(2):
                qk_copy(oi, 0, pss[oi])
            for tt in range(JPI):
                v_finish(tt, vap[tt])

            # ---- rounds r = 0..3: attn pairs (p, r) with the next round's
            # projections and the previous round's out chunks as fillers ----
            for r in range(n_it):
                if r > 0:
                    for ot in (0, 1):
                        for u in qk_units(ot, r):
                            u()
                fillers = []
                for ot in range(2, n_qk):
                    fillers.extend(qk_units(ot, r))
                if r + 1 < n_it:
                    for tt in range(JPI * (r + 1), JPI * (r + 2)):
                        fillers.extend(v_units(tt))
                # out chunks ride in the LATE rounds (R3 is Scalar-bound:
                # the exp pacing leaves PE slack that these fill)
                if r == 2:
                    fillers.extend(
                        (lambda cc: lambda: out_chunk(0, cc))(c) for c in range(2 * JPI)
                    )
                if r == n_it - 1:
                    for rr in (1, 2):
                        fillers.extend(
                            (lambda rrr, cc: lambda: out_chunk(rrr, cc))(rr, c)
                            for c in range(2 * JPI)
                        )
                post_tts = None
                if r == n_it - 1:
                    # emit out(r) chunks for t-block gt one lt later, so the
                    # chunk's p=3 matmul never head-blocks the PE queue on
                    # the just-enqueued DMA transpose
                    def final_post(lt):
                        if lt >= 1:
                            tt = JPI * r + lt - 1
                            for oh in range(n_oh):
                                out_chunk(r, (tt - JPI * r) * n_oh + oh)

                    post_tts = {n_pairs - 1: final_post}
                run_pairs([(p, r) for p in range(n_pairs)], fillers, post_tts)

            # final: the last t-block's out chunks
            for oh in range(n_oh):
                out_chunk(n_it - 1, (JPI - 1) * n_oh + oh)

    nc.compile()
    return nc


def make_tri_square(ts=TS):
    """[128, 2*ts] {0,1} keep-mask, duplicated per head: cell (s, t) = 0 iff
    s > t (strictly below the diagonal of the 128x128 square)."""
    s = np.arange(128)[:, None]
    t = np.arange(ts)[None, :]
    one = np.where(s > t, 0.0, 1.0).astype(np.float32)
    return np.concatenate([one, one], axis=1)


def make_core_inputs(x_b, W_qkv, b_qkv, W_out, heads, C_sz=C, T_sz=T):
    """Build the per-core input map (numpy, host-side)."""
    n_pairs = len(heads) // 2
    n_qk = 2 * n_pairs
    VW = len(heads) * DK
    xT = np.ascontiguousarray(x_b.T).astype(np.float32)
    wqk = np.empty((C_sz, n_qk * 128), np.float32)
    bqk = np.empty((128, n_qk), np.float32)
    wv = np.empty((C_sz, VW), np.float32)
    bv = np.empty((1, VW), np.float32)
    wo = np.empty((n_pairs * 128, C_sz), np.float32)
    for p in range(n_pairs):
        hA, hB = heads[2 * p], heads[2 * p + 1]
        # q tile (scaled by 1/sqrt(dk)=1/8), k tile
        for half, h in ((0, hA), (1, hB)):
            r0 = h * 3 * DK
            wqk[:, 2 * p * 128 + half * 64 : 2 * p * 128 + half * 64 + 64] = (
                W_qkv[r0 : r0 + DK].T / math.sqrt(DK)
            )
            bqk[half * 64 : half * 64 + 64, 2 * p] = b_qkv[r0 : r0 + DK] / math.sqrt(DK)
            wqk[:, (2 * p + 1) * 128 + half * 64 : (2 * p + 1) * 128 + half * 64 + 64] = (
                W_qkv[r0 + DK : r0 + 2 * DK].T
            )
            bqk[half * 64 : half * 64 + 64, 2 * p + 1] = b_qkv[r0 + DK : r0 + 2 * DK]
            wo[p * 128 + half * 64 : p * 128 + half * 64 + 64, :] = W_out[
                :, h * DK : (h + 1) * DK
            ].T
    for hh, h in enumerate(heads):
        r0 = h * 3 * DK + 2 * DK
        wv[:, hh * DK : (hh + 1) * DK] = W_qkv[r0 : r0 + DK].T
        bv[0, hh * DK : (hh + 1) * DK] = b_qkv[r0 : r0 + DK]
    return {
        "xT": xT.astype(BF16),
        "wqkA": np.ascontiguousarray(wqk[:, 0:256]).astype(BF16),
        "wqkB": np.ascontiguousarray(wqk[:, 256:]).astype(BF16),
        "wvT": wv.astype(BF16),
        "bqk": bqk.astype(np.float32),
        "bv": bv.astype(BF16),
        "woT": wo.astype(BF16),
        "trisq": make_tri_square().astype(BF16),
        "ident": np.eye(128, dtype=np.float32).astype(BF16),
    }


_NC_CACHE = {}


def kernel(x, W_qkv, b_qkv, W_out, b_out, _trace=False):
    x = np.asarray(x, dtype=np.float32)
    W_qkv = np.asarray(W_qkv, dtype=np.float32)
    b_qkv = np.asarray(b_qkv, dtype=np.float32)
    W_out = np.asarray(W_out, dtype=np.float32)
    b_out = np.asarray(b_out, dtype=np.float32)

    from concourse.bass_utils import run_bass_kernel_spmd

    key = ("full", C, T, 4)
    if key not in _NC_CACHE:
        _NC_CACHE[key] = build_program(C, T, n_pairs=4, num_devices=1)
    nc = _NC_CACHE[key]

    in_maps = []
    for core in range(NCORES):
        b, hg = divmod(core, 2)
        heads = list(range(hg * 8, hg * 8 + 8))
        in_maps.append(make_core_inputs(x[b], W_qkv, b_qkv, W_out, heads))

    res = run_bass_kernel_spmd(nc, in_maps, list(range(NCORES)), trace=_trace)
    kernel._last_results = res

    out = np.broadcast_to(b_out, (B, T, C)).astype(np.float32).copy()
    for core in range(NCORES):
        b = core // 2
        out[b] += res.results[core]["out"].astype(np.float32)
    return out
